# revision 36
# baseline (speedup 1.0000x reference)
"""MoE (dense-activated, 32 experts) Trainium2 kernel, v3.

Problem: out[b,t,u] = sum_e gate[b,t,e] * LeakyReLU((x @ We[e] + be[e]))[u]
         gate = x @ Wg + bg   (no softmax)
Shapes: x[32,512,128], Wg[128,32], bg[32], We[32,128,64], be[32,64] -> out[32,512,64]

Strategy: data-parallel over batch across 8 NeuronCores (4 batches = 2048
tokens per core), weights replicated, no collectives. Host pre-transposes
x so the contraction dim F=128 lands on SBUF partitions; x and weights
ship as ONE concatenated bf16 tensor [128, 4160] = [Wg-paired | xT | We].

v3 design (fp32 PSUM -- TRN2 matmul cannot write 16-bit PSUM):
- Gate matmuls land in short-lived PSUM tiles sharing the h rotation
  slots (tag slots rotate by allocation order), evacuated by ACT Copy:
  chunk A (tiles 0-3) up-front, chunk B after tile GATES_AFTER=1 --
  emitting B up-front wins ~2.6k in LOOP throughput but loses ~2.5k in
  SINGLE-SHOT span (B waits on the late x4-15 DMA while holding PSUM
  slots); the harness measures single-shot, so B stays mid-stream.
- Per tile: 4 PE matmuls [128,512] -> one 4-bank fp32 PSUM tile,
  ONE ACT Prelu [P,2048] -> SBUF bf16 (edge tiles split in halves),
  DVE: 2x-mode paired-gate multiply + 2-level add tree truncated at
  OUT_W=512 (software-pipelined: mult(i), tree(i-1)). Host folds the
  remaining 8 expert groups at gather time.
- Preload: SP ring carries [Wg|x0-3]; ACT ring carries We (split) then
  x4-15, so We lands before the late x tiles in DMA service order.

Measured on HW (R-slope, same-session comparisons; absolute numbers
drift +-10us with device contention -- compare only within a session):
- engine floors: PE 9.2us, PE+ACT 26.2us, DVE-iso ~40.7us
- best composed: 31.8-37.4k ns/sweep vs 45.9k for v2 baseline
- ablations REMOVING DVE work ran SLOWER (latency/resonance-bound
  regime, not throughput-bound): tune by measurement, not cost model.

Measured dead ends -- do NOT revisit:
- GPSIMD/Pool for any tree level: +1.1us/op of serialized handoff even
  in DVE isolation (L1_POOL=1 composed 63k vs 42k without).
- Grouped DVE ops (GROUPS=2/4/8): iso-pairs LOSE the 2x mode on HW
  (54.5k vs 40.7k singles); composed all worse than singles.
- PSUM_HALF (4x 2-bank PSUM slots, 2 Prelus/tile): ~3-4k worse paired.
- PIPE=2 (deeper sw-pipeline): worse (50.7k vs 31.8k adjacent).
- More hl/o_t buffers (HLP_BUFS=6/8): catastrophic (58-71k). Keep 4/4.
- gates via DVE tensor_copy (GATE_DVE=1): worse than ACT copies.
- tensor_reduce for the fold: no DVE fast modes (1x only) + fp32-out.
- TRN2 matmul must write fp32 PSUM (bf16 PSUM is TRN3+ only).
- OUT_W=1024/2048: output-DMA (HBM-write) bound, 55-78us sweeps (v2).
"""

import os
import sys

import numpy as np

for _p in ("/opt/trn_rl_repo", os.path.expanduser("~/.axon_site/_ro/trn_rl_repo")):
    if os.path.isdir(_p) and _p not in sys.path:
        sys.path.insert(0, _p)

import concourse.bass as bass
import concourse.bacc as bacc
import concourse.tile as tile
from concourse import mybir
from concourse.bass_utils import run_bass_kernel_spmd

ALPHA = 0.01

B, T, F, U, E = 32, 512, 128, 64, 32
N_CORES = 8
TOK = (B // N_CORES) * T          # tokens per core = 2048
P = 128                           # tokens per tile
N_TILES = TOK // P                # 16
EU = E * U                        # 2048

f32 = mybir.dt.float32
bf16 = mybir.dt.bfloat16

# Device add-tree stops at this width; the remaining OUT_W//U-way
# expert-group fold happens on the host during gather.
OUT_W = int(os.environ.get("OUT_W", "512"))
VAR = os.environ.get("VAR", "all")  # all|pe|act|dve (timing isolation)
A_TILES = int(os.environ.get("A_TILES", "4"))   # x tiles in the first DMA
WE_SPLIT = int(os.environ.get("WE_SPLIT", "1"))  # split the We DMA in half
HEAD_HALVES = int(os.environ.get("HEAD_HALVES", "2"))  # first N tiles: 2 Prelus
GATE_DVE = int(os.environ.get("GATE_DVE", "0"))  # gate evac on DVE (else ACT)
L1_POOL = int(os.environ.get("L1_POOL", "0"))   # 0 never / 1 alternate / 2 always
TAIL_DVE = int(os.environ.get("TAIL_DVE", "1"))  # last group's L1 on DVE
HLP_BUFS = int(os.environ.get("HLP_BUFS", "4"))
OUTP_BUFS = int(os.environ.get("OUTP_BUFS", "4"))
# PSUM_HALF: h PSUM tiles are [P,1024] (2 banks) -> 4-slot rotation,
# one Prelu per half. Doubles PE<->ACT pipeline depth.
PSUM_HALF = int(os.environ.get("PSUM_HALF", "0"))
PH_BUFS = int(os.environ.get("PH_BUFS", "4" if PSUM_HALF else "2"))
# Gate chunk B placement: -1 = all gates upfront (best loop throughput,
# but in SINGLE-SHOT the upfront chunks wait on the late x4-15 DMA while
# holding PSUM slots, delaying the first Prelus ~2.5us); N>=0 = emit
# after tile N's h-work (best single-shot span, which is what the
# harness measures).
GATES_AFTER = int(os.environ.get("GATES_AFTER", "1"))
# DVE op granularity: comma list of group sizes summing to 16. DVE
# mult/L0/L1 each cover a whole group in ONE instruction (composite
# (tile,expert) dim has uniform stride 64), amortizing per-instruction
# overhead. Bigger groups = fewer instrs but later start + longer tail.
GROUPS = [int(g) for g in os.environ.get(
    "GROUPS", ",".join(["1"] * N_TILES)).split(",")]
assert sum(GROUPS) == N_TILES
# timing-only ablations (produce wrong results; for cost decomposition)
TREE = int(os.environ.get("TREE", "2"))   # 2=L0+L1, 1=L0 only, 0=none
MULT = int(os.environ.get("MULT", "1"))   # 0 skips the gate multiply
# software-pipeline depth: 1 = [mult(i), tree(i-1)];
# 2 = [mult(i), L0(i-1), L1+dma(i-2)]
PIPE = int(os.environ.get("PIPE", "1"))
# split the LAST tile's gate-multiply per Prelu half so the tail DVE
# chain starts one half-Prelu earlier (single-shot tail shave)
TAIL_SPLIT = int(os.environ.get("TAIL_SPLIT", "0"))
# output-DMA dispatch ring: sync | alt (alternate SP/ACT rings per tile)
OUT_RING = os.environ.get("OUT_RING", "sync")

GOFF = 0
XOFF = 2 * E                      # 64
HOFF = XOFF + TOK                 # 2112
W_COLS = HOFF + EU                # 4160

_CACHED = {}


def _xt_col(i):
    return XOFF + i * P


def _build_nc(reps=1):
    """Build the single-core SPMD Bass module."""
    from contextlib import ExitStack

    nc = bacc.Bacc("TRN2")
    XW = nc.declare_dram_parameter("XW", [F, W_COLS], bf16, isOutput=False)
    o_dt = f32 if OUT_W == U else bf16
    O = nc.declare_dram_parameter("O", [TOK, OUT_W], o_dt, isOutput=True)

    with ExitStack() as ctx:
        tc = ctx.enter_context(tile.TileContext(nc))
        singles = ctx.enter_context(tc.tile_pool(name="singles", bufs=1))
        gsb = ctx.enter_context(tc.tile_pool(name="gsb", bufs=2))
        hlp = ctx.enter_context(tc.tile_pool(name="hlp", bufs=HLP_BUFS))
        outp = ctx.enter_context(tc.tile_pool(name="outp", bufs=OUTP_BUFS))
        php = ctx.enter_context(tc.tile_pool(name="php", bufs=PH_BUFS,
                                             space="PSUM"))

        xw = singles.tile([F, W_COLS], bf16)

        A_COLS = XOFF + A_TILES * P

        def emit_preload():
            # ring 0 (SP): gates' + first tiles' x lands first
            # ring 1 (ACT): We (split for tile 0's first matmul), then the
            # rest of x — keeping x4-15 BEHIND We in DMA service order
            nc.sync.dma_start(out=xw[:, 0:A_COLS], in_=XW[:, 0:A_COLS])
            if WE_SPLIT:
                n = 2 ** WE_SPLIT       # 1 -> halves, 2 -> quarters
                step = EU // n
                for k in range(n):
                    c0 = HOFF + k * step
                    nc.scalar.dma_start(out=xw[:, c0:c0 + step],
                                        in_=XW[:, c0:c0 + step])
            else:
                nc.scalar.dma_start(out=xw[:, HOFF:], in_=XW[:, HOFF:])
            q0b_eng = nc.scalar if os.environ.get("Q0B", "scalar") == "scalar" \
                else nc.sync
            q0b_eng.dma_start(out=xw[:, A_COLS:HOFF], in_=XW[:, A_COLS:HOFF])

        emit_preload()

        def emit_sweep():
            do_pe = VAR in ("all", "pe", "act")
            do_act = VAR in ("all", "act")
            do_dve = VAR in ("all", "dve")

            # ---- gates: two short-lived fp32 PSUM tiles (share php slots)
            g2 = gsb.tile([P, N_TILES * 2 * E], bf16)   # [p, (i, e, 2)]

            def gate_chunk(i0, i1):
                # matmul gates for tiles [i0, i1) then evacuate; the PSUM
                # tile is released immediately so h tiles can rotate
                g_ps = php.tile([P, (i1 - i0) * 2 * E], f32, tag="ps")
                for i in range(i0, i1):
                    nc.tensor.matmul(
                        g_ps[:, (i - i0) * 64:(i - i0 + 1) * 64],
                        lhsT=xw[:, _xt_col(i):_xt_col(i) + P],
                        rhs=xw[:, GOFF:GOFF + 2 * E],
                        start=True, stop=True,
                    )
                if GATE_DVE:
                    nc.vector.tensor_copy(g2[:, i0 * 64:i1 * 64], g_ps[:])
                else:
                    nc.scalar.activation(
                        g2[:, i0 * 64:i1 * 64], g_ps[:],
                        mybir.ActivationFunctionType.Copy)

            if VAR == "dve":
                nc.vector.tensor_copy(g2[:], xw[:, 0:N_TILES * 2 * E])
            if VAR == "all":
                # both chunks allocated up-front: the PSUM slot churn lands
                # in the idle head, before the h rotation starts. Chunk B's
                # matmuls wait on the x4-15 DMA, its copy slots into ACT
                # between early Prelus.
                gate_chunk(0, A_TILES)
                if GATES_AFTER < 0:
                    if PSUM_HALF:
                        # keep each gate PSUM tile <= 512 fp32 cols so the
                        # shared tag slot stays at 1 PSUM bank
                        b = min(A_TILES + 8, N_TILES)
                        gate_chunk(A_TILES, b)
                        if b < N_TILES:
                            gate_chunk(b, N_TILES)
                    else:
                        gate_chunk(A_TILES, N_TILES)

            # ---- per-tile h pipeline; DVE ops cover GROUPS of tiles ----
            pend = None

            def emit_mult(hl, i0, G, half=None):
                # 2x-mode multiply hl *= gate (pair-packed); (tile,e) is a
                # single uniform-stride-64 dim so one op covers the group.
                # half=0/1 covers one 1024-col half (experts 0-15/16-31).
                if half is None:
                    hs, gs, ne = hl, g2[:, i0 * 64:(i0 + G) * 64], G * E
                else:
                    hs = hl[:, half * 1024:(half + 1) * 1024]
                    gs = g2[:, i0 * 64 + half * 32:i0 * 64 + (half + 1) * 32]
                    ne = E // 2
                h4 = hs.rearrange("p (te u2 two) -> p te u2 two",
                                  te=ne, two=2)
                g24 = (gs.rearrange("p (te two) -> p te two", two=2)
                       .unsqueeze(2)
                       .broadcast_to([P, ne, U // 2, 2]))
                nc.vector.tensor_tensor(h4, h4, g24,
                                        op=mybir.AluOpType.mult)

            def emit_tree(hl, i0, G):
                # grouped expert add-tree, truncated at OUT_W
                w = EU // 2
                lvl = 0
                while w > OUT_W:
                    lvl += 1
                    if TREE >= lvl:
                        a = hl.rearrange("p (t c) -> p t c", t=G)
                        nc.vector.tensor_tensor(
                            a[:, :, 0:w], a[:, :, 0:w], a[:, :, w:2 * w],
                            op=mybir.AluOpType.add)
                    w //= 2
                o_t = outp.tile([P, G * OUT_W], o_dt)
                ov = o_t.rearrange("p (t c) -> p t c", t=G)
                a = hl.rearrange("p (t c) -> p t c", t=G)
                eng = nc.vector
                if L1_POOL == 2 or (L1_POOL == 1 and (i0 // max(GROUPS)) % 2):
                    eng = nc.gpsimd
                if TAIL_DVE and i0 + G == N_TILES:
                    eng = nc.vector
                if TREE >= lvl + 1:
                    eng.tensor_tensor(
                        ov[:], a[:, :, 0:w], a[:, :, w:2 * w],
                        op=mybir.AluOpType.add)
                    for t in range(G):
                        _out_eng(i0 + t).dma_start(
                            out=O[(i0 + t) * P:(i0 + t + 1) * P, :],
                            in_=o_t[:, t * OUT_W:(t + 1) * OUT_W])
                else:
                    # timing ablation: ship (wrong) partials straight out
                    for t in range(G):
                        _out_eng(i0 + t).dma_start(
                            out=O[(i0 + t) * P:(i0 + t + 1) * P, :],
                            in_=hl[:, t * EU:t * EU + OUT_W])

            def _out_eng(i):
                if OUT_RING == "alt":
                    return nc.scalar if i % 2 else nc.sync
                return nc.sync

            def emit_l0(hl, i0, G):
                if TREE >= 1:
                    a = hl.rearrange("p (t c) -> p t c", t=G)
                    nc.vector.tensor_tensor(
                        a[:, :, 0:1024], a[:, :, 0:1024], a[:, :, 1024:2048],
                        op=mybir.AluOpType.add)

            def emit_l1(hl, i0, G):
                a = hl.rearrange("p (t c) -> p t c", t=G)
                if TREE >= 2:
                    o_t = outp.tile([P, G * OUT_W], o_dt)
                    ov = o_t.rearrange("p (t c) -> p t c", t=G)
                    nc.vector.tensor_tensor(
                        ov[:], a[:, :, 0:512], a[:, :, 512:1024],
                        op=mybir.AluOpType.add)
                    for t in range(G):
                        nc.sync.dma_start(
                            out=O[(i0 + t) * P:(i0 + t + 1) * P, :],
                            in_=o_t[:, t * OUT_W:(t + 1) * OUT_W])
                else:
                    for t in range(G):
                        nc.sync.dma_start(
                            out=O[(i0 + t) * P:(i0 + t + 1) * P, :],
                            in_=hl[:, t * EU:t * EU + OUT_W])

            p0 = p1 = None
            i0 = 0
            for G in GROUPS:
                hl = hlp.tile([P, G * EU], bf16)
                for t in range(G):
                    i = i0 + t
                    xt_r = xw[:, _xt_col(i):_xt_col(i) + P]
                    hs = hl[:, t * EU:(t + 1) * EU]
                    halves = 2 if (i < HEAD_HALVES or i == N_TILES - 1) else 1
                    if do_pe and PSUM_HALF:
                        for half in range(2):
                            h_ps = php.tile([P, EU // 2], f32, tag="ps")
                            for j in range(2):
                                c0 = HOFF + half * 1024 + j * 512
                                nc.tensor.matmul(
                                    h_ps[:, j * 512:(j + 1) * 512],
                                    lhsT=xt_r,
                                    rhs=xw[:, c0:c0 + 512],
                                    start=True, stop=True,
                                )
                            if do_act:
                                nc.scalar.activation(
                                    hs[:, half * 1024:(half + 1) * 1024],
                                    h_ps[:],
                                    mybir.ActivationFunctionType.Prelu,
                                    alpha=ALPHA)
                    elif do_pe:
                        h_ps = php.tile([P, EU], f32, tag="ps")
                        for j in range(4):
                            nc.tensor.matmul(
                                h_ps[:, j * 512:(j + 1) * 512],
                                lhsT=xt_r,
                                rhs=xw[:, HOFF + j * 512:HOFF + (j + 1) * 512],
                                start=True, stop=True,
                            )
                            if do_act and halves == 2 and j % 2 == 1:
                                nc.scalar.activation(
                                    hs[:, (j - 1) * 512:(j + 1) * 512],
                                    h_ps[:, (j - 1) * 512:(j + 1) * 512],
                                    mybir.ActivationFunctionType.Prelu,
                                    alpha=ALPHA)
                        if do_act and halves == 1:
                            nc.scalar.activation(
                                hs[:], h_ps[:],
                                mybir.ActivationFunctionType.Prelu,
                                alpha=ALPHA)
                    if i == GATES_AFTER and VAR == "all":
                        gate_chunk(A_TILES, N_TILES)
                if do_dve:
                    if MULT and TAIL_SPLIT and G == 1 and i0 == N_TILES - 1:
                        # tree(14) fills the DVE wait for Prelu15's 2nd half
                        emit_mult(hl, i0, G, half=0)
                        if pend is not None:
                            emit_tree(*pend)
                            pend = None
                        emit_mult(hl, i0, G, half=1)
                    elif MULT:
                        emit_mult(hl, i0, G)
                    if PIPE == 2 and OUT_W == 512:
                        if p0 is not None:
                            emit_l0(*p0)
                        if p1 is not None:
                            emit_l1(*p1)
                        p1 = p0
                        p0 = (hl, i0, G)
                    else:
                        if pend is not None:
                            emit_tree(*pend)
                        pend = (hl, i0, G)
                i0 += G
            if do_dve and PIPE == 2 and OUT_W == 512:
                if p0 is not None:
                    emit_l0(*p0)
                if p1 is not None:
                    emit_l1(*p1)
                if p0 is not None:
                    emit_l1(*p0)
            elif do_dve and pend is not None:
                emit_tree(*pend)

        if reps == 1:
            emit_sweep()
        else:
            with tc.For_i(0, reps, 1):
                emit_sweep()

    nc.finalize()
    return nc


def _numpy_fallback(x, Wg, bg, We, be):
    gate = np.einsum("btf,fe->bte", x, Wg) + bg
    h = np.einsum("btf,efu->btue", x, We) + be.T
    h = np.where(h >= 0, h, ALPHA * h)
    return np.einsum("btue,bte->btu", h, gate).astype(np.float32)


LAST_RESULTS = None


def prepare_in_maps(x, Wg, bg, We, be):
    # XW = [Wg-paired(64) | xT(2048) | We_flat(2048)]
    import ml_dtypes
    Wg2 = np.repeat(Wg, 2, axis=1)
    We_f = We.transpose(1, 0, 2).reshape(F, E * U)
    xs = x.reshape(N_CORES, TOK, F)
    maps = []
    for c in range(N_CORES):
        xT = xs[c].T  # [F, TOK]
        maps.append({"XW": np.ascontiguousarray(np.concatenate(
            [Wg2, xT, We_f], axis=1
        ).astype(ml_dtypes.bfloat16))})
    return maps


def kernel(x, Wg, bg, We, be):
    x = np.asarray(x, dtype=np.float32)
    Wg = np.asarray(Wg, dtype=np.float32)
    bg = np.asarray(bg, dtype=np.float32)
    We = np.asarray(We, dtype=np.float32)
    be = np.asarray(be, dtype=np.float32)

    # device fast path assumes zero biases (true for this problem's inputs)
    if np.any(bg) or np.any(be):
        return _numpy_fallback(x, Wg, bg, We, be)

    if "nc" not in _CACHED:
        _CACHED["nc"] = _build_nc()
    nc = _CACHED["nc"]

    in_maps = prepare_in_maps(x, Wg, bg, We, be)

    global LAST_RESULTS
    res = run_bass_kernel_spmd(nc, in_maps, list(range(N_CORES)))
    LAST_RESULTS = res
    out = np.stack([np.asarray(res.results[c]["O"]) for c in range(N_CORES)],
                   axis=0)
    if OUT_W != U:
        # gather-time fold of the remaining expert groups (fast bf16 ->
        # fp32 upcast via bit trick, then reduce)
        if out.dtype != np.float32:
            out = (out.view(np.uint16).astype(np.uint32) << 16).view(
                np.float32)
        out = out.reshape(N_CORES, TOK, OUT_W // U, U).sum(axis=2)
    return out.reshape(B, T, U).astype(np.float32)


# revision 37
# speedup vs baseline: 1.1233x; 1.1233x over previous
"""MoE (dense-activated, 32 experts) Trainium2 kernel, v3.

Problem: out[b,t,u] = sum_e gate[b,t,e] * LeakyReLU((x @ We[e] + be[e]))[u]
         gate = x @ Wg + bg   (no softmax)
Shapes: x[32,512,128], Wg[128,32], bg[32], We[32,128,64], be[32,64] -> out[32,512,64]

Strategy: data-parallel over batch across 8 NeuronCores (4 batches = 2048
tokens per core), weights replicated, no collectives. Host pre-transposes
x so the contraction dim F=128 lands on SBUF partitions; x and weights
ship as ONE concatenated bf16 tensor [128, 4160] = [Wg-paired | xT | We].

v3 design (fp32 PSUM -- TRN2 matmul cannot write 16-bit PSUM):
- Gate matmuls land in short-lived PSUM tiles sharing the h rotation
  slots (tag slots rotate by allocation order), evacuated by ACT Copy:
  chunk A (tiles 0-3) up-front, chunk B after tile GATES_AFTER=1 --
  emitting B up-front wins ~2.6k in LOOP throughput but loses ~2.5k in
  SINGLE-SHOT span (B waits on the late x4-15 DMA while holding PSUM
  slots); the harness measures single-shot, so B stays mid-stream.
- Per tile: 4 PE matmuls [128,512] -> one 4-bank fp32 PSUM tile,
  ONE ACT Prelu [P,2048] -> SBUF bf16 (edge tiles split in halves),
  DVE: 2x-mode paired-gate multiply + 2-level add tree truncated at
  OUT_W=512 (software-pipelined: mult(i), tree(i-1)). Host folds the
  remaining 8 expert groups at gather time.
- Preload: SP ring carries [Wg|x0-3]; ACT ring carries We (split) then
  x4-15, so We lands before the late x tiles in DMA service order.

Measured on HW (R-slope, same-session comparisons; absolute numbers
drift +-10us with device contention -- compare only within a session):
- engine floors: PE 9.2us, PE+ACT 26.2us, DVE-iso ~40.7us
- best composed: 31.8-37.4k ns/sweep vs 45.9k for v2 baseline
- ablations REMOVING DVE work ran SLOWER (latency/resonance-bound
  regime, not throughput-bound): tune by measurement, not cost model.

Measured dead ends -- do NOT revisit:
- GPSIMD/Pool for any tree level: +1.1us/op of serialized handoff even
  in DVE isolation (L1_POOL=1 composed 63k vs 42k without).
- Grouped DVE ops (GROUPS=2/4/8): iso-pairs LOSE the 2x mode on HW
  (54.5k vs 40.7k singles); composed all worse than singles.
- PSUM_HALF (4x 2-bank PSUM slots, 2 Prelus/tile): ~3-4k worse paired.
- PIPE=2 (deeper sw-pipeline): worse (50.7k vs 31.8k adjacent).
- More hl/o_t buffers (HLP_BUFS=6/8): catastrophic (58-71k). Keep 4/4.
- gates via DVE tensor_copy (GATE_DVE=1): statistical tie with ACT
  copies under high-R interleaved A/B (not a confirmed dead end, but
  not a win either; ACT copies kept as the verified default).
- tensor_reduce for the fold: no DVE fast modes (1x only) + fp32-out.
- TRN2 matmul must write fp32 PSUM (bf16 PSUM is TRN3+ only).
- OUT_W=1024/2048: output-DMA (HBM-write) bound, 55-78us sweeps (v2).
"""

import os
import sys

import numpy as np

for _p in ("/opt/trn_rl_repo", os.path.expanduser("~/.axon_site/_ro/trn_rl_repo")):
    if os.path.isdir(_p) and _p not in sys.path:
        sys.path.insert(0, _p)

import concourse.bass as bass
import concourse.bacc as bacc
import concourse.tile as tile
from concourse import mybir
from concourse.bass_utils import run_bass_kernel_spmd

ALPHA = 0.01

B, T, F, U, E = 32, 512, 128, 64, 32
N_CORES = 8
TOK = (B // N_CORES) * T          # tokens per core = 2048
P = 128                           # tokens per tile
N_TILES = TOK // P                # 16
EU = E * U                        # 2048

f32 = mybir.dt.float32
bf16 = mybir.dt.bfloat16

# Device add-tree stops at this width; the remaining OUT_W//U-way
# expert-group fold happens on the host during gather.
OUT_W = int(os.environ.get("OUT_W", "512"))
VAR = os.environ.get("VAR", "all")  # all|pe|act|dve (timing isolation)
A_TILES = int(os.environ.get("A_TILES", "4"))   # x tiles in the first DMA
WE_SPLIT = int(os.environ.get("WE_SPLIT", "1"))  # split the We DMA in half
HEAD_HALVES = int(os.environ.get("HEAD_HALVES", "2"))  # first N tiles: 2 Prelus
GATE_DVE = int(os.environ.get("GATE_DVE", "0"))  # gate evac on DVE (else ACT)
L1_POOL = int(os.environ.get("L1_POOL", "0"))   # 0 never / 1 alternate / 2 always
TAIL_DVE = int(os.environ.get("TAIL_DVE", "1"))  # last group's L1 on DVE
HLP_BUFS = int(os.environ.get("HLP_BUFS", "4"))
OUTP_BUFS = int(os.environ.get("OUTP_BUFS", "4"))
# PSUM_HALF: h PSUM tiles are [P,1024] (2 banks) -> 4-slot rotation,
# one Prelu per half. Doubles PE<->ACT pipeline depth.
PSUM_HALF = int(os.environ.get("PSUM_HALF", "0"))
PH_BUFS = int(os.environ.get("PH_BUFS", "4" if PSUM_HALF else "2"))
# Gate chunk B placement: -1 = all gates upfront (best loop throughput,
# but in SINGLE-SHOT the upfront chunks wait on the late x4-15 DMA while
# holding PSUM slots, delaying the first Prelus ~2.5us); N>=0 = emit
# after tile N's h-work (best single-shot span, which is what the
# harness measures).
GATES_AFTER = int(os.environ.get("GATES_AFTER", "1"))
# DVE op granularity: comma list of group sizes summing to 16. DVE
# mult/L0/L1 each cover a whole group in ONE instruction (composite
# (tile,expert) dim has uniform stride 64), amortizing per-instruction
# overhead. Bigger groups = fewer instrs but later start + longer tail.
GROUPS = [int(g) for g in os.environ.get(
    "GROUPS", ",".join(["1"] * N_TILES)).split(",")]
assert sum(GROUPS) == N_TILES
# timing-only ablations (produce wrong results; for cost decomposition)
TREE = int(os.environ.get("TREE", "2"))   # 2=L0+L1, 1=L0 only, 0=none
MULT = int(os.environ.get("MULT", "1"))   # 0 skips the gate multiply
# software-pipeline depth: 1 = [mult(i), tree(i-1)];
# 2 = [mult(i), L0(i-1), L1+dma(i-2)]
PIPE = int(os.environ.get("PIPE", "1"))
# split the LAST tile's gate-multiply per Prelu half so the tail DVE
# chain starts one half-Prelu earlier (single-shot tail shave)
TAIL_SPLIT = int(os.environ.get("TAIL_SPLIT", "0"))
# output-DMA dispatch ring: sync | alt (alternate SP/ACT rings per tile)
OUT_RING = os.environ.get("OUT_RING", "sync")

GOFF = 0
XOFF = 2 * E                      # 64
HOFF = XOFF + TOK                 # 2112
W_COLS = HOFF + EU                # 4160

_CACHED = {}


def _xt_col(i):
    return XOFF + i * P


def _build_nc(reps=1):
    """Build the single-core SPMD Bass module."""
    from contextlib import ExitStack

    nc = bacc.Bacc("TRN2")
    XW = nc.declare_dram_parameter("XW", [F, W_COLS], bf16, isOutput=False)
    o_dt = f32 if OUT_W == U else bf16
    O = nc.declare_dram_parameter("O", [TOK, OUT_W], o_dt, isOutput=True)

    with ExitStack() as ctx:
        tc = ctx.enter_context(tile.TileContext(nc))
        singles = ctx.enter_context(tc.tile_pool(name="singles", bufs=1))
        gsb = ctx.enter_context(tc.tile_pool(name="gsb", bufs=2))
        hlp = ctx.enter_context(tc.tile_pool(name="hlp", bufs=HLP_BUFS))
        outp = ctx.enter_context(tc.tile_pool(name="outp", bufs=OUTP_BUFS))
        php = ctx.enter_context(tc.tile_pool(name="php", bufs=PH_BUFS,
                                             space="PSUM"))

        xw = singles.tile([F, W_COLS], bf16)

        A_COLS = XOFF + A_TILES * P

        def emit_preload():
            # ring 0 (SP): gates' + first tiles' x lands first
            # ring 1 (ACT): We (split for tile 0's first matmul), then the
            # rest of x — keeping x4-15 BEHIND We in DMA service order
            nc.sync.dma_start(out=xw[:, 0:A_COLS], in_=XW[:, 0:A_COLS])
            if WE_SPLIT:
                n = 2 ** WE_SPLIT       # 1 -> halves, 2 -> quarters
                step = EU // n
                for k in range(n):
                    c0 = HOFF + k * step
                    nc.scalar.dma_start(out=xw[:, c0:c0 + step],
                                        in_=XW[:, c0:c0 + step])
            else:
                nc.scalar.dma_start(out=xw[:, HOFF:], in_=XW[:, HOFF:])
            q0b_eng = nc.scalar if os.environ.get("Q0B", "scalar") == "scalar" \
                else nc.sync
            q0b_eng.dma_start(out=xw[:, A_COLS:HOFF], in_=XW[:, A_COLS:HOFF])

        emit_preload()

        def emit_sweep():
            do_pe = VAR in ("all", "pe", "act")
            do_act = VAR in ("all", "act")
            do_dve = VAR in ("all", "dve")

            # ---- gates: two short-lived fp32 PSUM tiles (share php slots)
            g2 = gsb.tile([P, N_TILES * 2 * E], bf16)   # [p, (i, e, 2)]

            def gate_chunk(i0, i1):
                # matmul gates for tiles [i0, i1) then evacuate; the PSUM
                # tile is released immediately so h tiles can rotate
                g_ps = php.tile([P, (i1 - i0) * 2 * E], f32, tag="ps")
                for i in range(i0, i1):
                    nc.tensor.matmul(
                        g_ps[:, (i - i0) * 64:(i - i0 + 1) * 64],
                        lhsT=xw[:, _xt_col(i):_xt_col(i) + P],
                        rhs=xw[:, GOFF:GOFF + 2 * E],
                        start=True, stop=True,
                    )
                if GATE_DVE:
                    nc.vector.tensor_copy(g2[:, i0 * 64:i1 * 64], g_ps[:])
                else:
                    nc.scalar.activation(
                        g2[:, i0 * 64:i1 * 64], g_ps[:],
                        mybir.ActivationFunctionType.Copy)

            if VAR == "dve":
                nc.vector.tensor_copy(g2[:], xw[:, 0:N_TILES * 2 * E])
            if VAR == "all":
                # both chunks allocated up-front: the PSUM slot churn lands
                # in the idle head, before the h rotation starts. Chunk B's
                # matmuls wait on the x4-15 DMA, its copy slots into ACT
                # between early Prelus.
                gate_chunk(0, A_TILES)
                if GATES_AFTER < 0:
                    if PSUM_HALF:
                        # keep each gate PSUM tile <= 512 fp32 cols so the
                        # shared tag slot stays at 1 PSUM bank
                        b = min(A_TILES + 8, N_TILES)
                        gate_chunk(A_TILES, b)
                        if b < N_TILES:
                            gate_chunk(b, N_TILES)
                    else:
                        gate_chunk(A_TILES, N_TILES)

            # ---- per-tile h pipeline; DVE ops cover GROUPS of tiles ----
            pend = None

            def emit_mult(hl, i0, G, half=None):
                # 2x-mode multiply hl *= gate (pair-packed); (tile,e) is a
                # single uniform-stride-64 dim so one op covers the group.
                # half=0/1 covers one 1024-col half (experts 0-15/16-31).
                if half is None:
                    hs, gs, ne = hl, g2[:, i0 * 64:(i0 + G) * 64], G * E
                else:
                    hs = hl[:, half * 1024:(half + 1) * 1024]
                    gs = g2[:, i0 * 64 + half * 32:i0 * 64 + (half + 1) * 32]
                    ne = E // 2
                h4 = hs.rearrange("p (te u2 two) -> p te u2 two",
                                  te=ne, two=2)
                g24 = (gs.rearrange("p (te two) -> p te two", two=2)
                       .unsqueeze(2)
                       .broadcast_to([P, ne, U // 2, 2]))
                nc.vector.tensor_tensor(h4, h4, g24,
                                        op=mybir.AluOpType.mult)

            def emit_tree(hl, i0, G):
                # grouped expert add-tree, truncated at OUT_W
                w = EU // 2
                lvl = 0
                while w > OUT_W:
                    lvl += 1
                    if TREE >= lvl:
                        a = hl.rearrange("p (t c) -> p t c", t=G)
                        nc.vector.tensor_tensor(
                            a[:, :, 0:w], a[:, :, 0:w], a[:, :, w:2 * w],
                            op=mybir.AluOpType.add)
                    w //= 2
                o_t = outp.tile([P, G * OUT_W], o_dt)
                ov = o_t.rearrange("p (t c) -> p t c", t=G)
                a = hl.rearrange("p (t c) -> p t c", t=G)
                eng = nc.vector
                if L1_POOL == 2 or (L1_POOL == 1 and (i0 // max(GROUPS)) % 2):
                    eng = nc.gpsimd
                if TAIL_DVE and i0 + G == N_TILES:
                    eng = nc.vector
                if TREE >= lvl + 1:
                    eng.tensor_tensor(
                        ov[:], a[:, :, 0:w], a[:, :, w:2 * w],
                        op=mybir.AluOpType.add)
                    for t in range(G):
                        _out_eng(i0 + t).dma_start(
                            out=O[(i0 + t) * P:(i0 + t + 1) * P, :],
                            in_=o_t[:, t * OUT_W:(t + 1) * OUT_W])
                else:
                    # timing ablation: ship (wrong) partials straight out
                    for t in range(G):
                        _out_eng(i0 + t).dma_start(
                            out=O[(i0 + t) * P:(i0 + t + 1) * P, :],
                            in_=hl[:, t * EU:t * EU + OUT_W])

            def _out_eng(i):
                if OUT_RING == "alt":
                    return nc.scalar if i % 2 else nc.sync
                return nc.sync

            def emit_l0(hl, i0, G):
                if TREE >= 1:
                    a = hl.rearrange("p (t c) -> p t c", t=G)
                    nc.vector.tensor_tensor(
                        a[:, :, 0:1024], a[:, :, 0:1024], a[:, :, 1024:2048],
                        op=mybir.AluOpType.add)

            def emit_l1(hl, i0, G):
                a = hl.rearrange("p (t c) -> p t c", t=G)
                if TREE >= 2:
                    o_t = outp.tile([P, G * OUT_W], o_dt)
                    ov = o_t.rearrange("p (t c) -> p t c", t=G)
                    nc.vector.tensor_tensor(
                        ov[:], a[:, :, 0:512], a[:, :, 512:1024],
                        op=mybir.AluOpType.add)
                    for t in range(G):
                        nc.sync.dma_start(
                            out=O[(i0 + t) * P:(i0 + t + 1) * P, :],
                            in_=o_t[:, t * OUT_W:(t + 1) * OUT_W])
                else:
                    for t in range(G):
                        nc.sync.dma_start(
                            out=O[(i0 + t) * P:(i0 + t + 1) * P, :],
                            in_=hl[:, t * EU:t * EU + OUT_W])

            p0 = p1 = None
            i0 = 0
            for G in GROUPS:
                hl = hlp.tile([P, G * EU], bf16)
                for t in range(G):
                    i = i0 + t
                    xt_r = xw[:, _xt_col(i):_xt_col(i) + P]
                    hs = hl[:, t * EU:(t + 1) * EU]
                    halves = 2 if (i < HEAD_HALVES or i == N_TILES - 1) else 1
                    if do_pe and PSUM_HALF:
                        for half in range(2):
                            h_ps = php.tile([P, EU // 2], f32, tag="ps")
                            for j in range(2):
                                c0 = HOFF + half * 1024 + j * 512
                                nc.tensor.matmul(
                                    h_ps[:, j * 512:(j + 1) * 512],
                                    lhsT=xt_r,
                                    rhs=xw[:, c0:c0 + 512],
                                    start=True, stop=True,
                                )
                            if do_act:
                                nc.scalar.activation(
                                    hs[:, half * 1024:(half + 1) * 1024],
                                    h_ps[:],
                                    mybir.ActivationFunctionType.Prelu,
                                    alpha=ALPHA)
                    elif do_pe:
                        h_ps = php.tile([P, EU], f32, tag="ps")
                        for j in range(4):
                            nc.tensor.matmul(
                                h_ps[:, j * 512:(j + 1) * 512],
                                lhsT=xt_r,
                                rhs=xw[:, HOFF + j * 512:HOFF + (j + 1) * 512],
                                start=True, stop=True,
                            )
                            if do_act and halves == 2 and j % 2 == 1:
                                nc.scalar.activation(
                                    hs[:, (j - 1) * 512:(j + 1) * 512],
                                    h_ps[:, (j - 1) * 512:(j + 1) * 512],
                                    mybir.ActivationFunctionType.Prelu,
                                    alpha=ALPHA)
                        if do_act and halves == 1:
                            nc.scalar.activation(
                                hs[:], h_ps[:],
                                mybir.ActivationFunctionType.Prelu,
                                alpha=ALPHA)
                    if i == GATES_AFTER and VAR == "all":
                        gate_chunk(A_TILES, N_TILES)
                if do_dve:
                    if MULT and TAIL_SPLIT and G == 1 and i0 == N_TILES - 1:
                        # tree(14) fills the DVE wait for Prelu15's 2nd half
                        emit_mult(hl, i0, G, half=0)
                        if pend is not None:
                            emit_tree(*pend)
                            pend = None
                        emit_mult(hl, i0, G, half=1)
                    elif MULT:
                        emit_mult(hl, i0, G)
                    if PIPE == 2 and OUT_W == 512:
                        if p0 is not None:
                            emit_l0(*p0)
                        if p1 is not None:
                            emit_l1(*p1)
                        p1 = p0
                        p0 = (hl, i0, G)
                    else:
                        if pend is not None:
                            emit_tree(*pend)
                        pend = (hl, i0, G)
                i0 += G
            if do_dve and PIPE == 2 and OUT_W == 512:
                if p0 is not None:
                    emit_l0(*p0)
                if p1 is not None:
                    emit_l1(*p1)
                if p0 is not None:
                    emit_l1(*p0)
            elif do_dve and pend is not None:
                emit_tree(*pend)

        if reps == 1:
            emit_sweep()
        else:
            with tc.For_i(0, reps, 1):
                emit_sweep()

    nc.finalize()
    return nc


def _numpy_fallback(x, Wg, bg, We, be):
    gate = np.einsum("btf,fe->bte", x, Wg) + bg
    h = np.einsum("btf,efu->btue", x, We) + be.T
    h = np.where(h >= 0, h, ALPHA * h)
    return np.einsum("btue,bte->btu", h, gate).astype(np.float32)


LAST_RESULTS = None


def prepare_in_maps(x, Wg, bg, We, be):
    # XW = [Wg-paired(64) | xT(2048) | We_flat(2048)]
    import ml_dtypes
    Wg2 = np.repeat(Wg, 2, axis=1)
    We_f = We.transpose(1, 0, 2).reshape(F, E * U)
    xs = x.reshape(N_CORES, TOK, F)
    maps = []
    for c in range(N_CORES):
        xT = xs[c].T  # [F, TOK]
        maps.append({"XW": np.ascontiguousarray(np.concatenate(
            [Wg2, xT, We_f], axis=1
        ).astype(ml_dtypes.bfloat16))})
    return maps


def kernel(x, Wg, bg, We, be):
    x = np.asarray(x, dtype=np.float32)
    Wg = np.asarray(Wg, dtype=np.float32)
    bg = np.asarray(bg, dtype=np.float32)
    We = np.asarray(We, dtype=np.float32)
    be = np.asarray(be, dtype=np.float32)

    # device fast path assumes zero biases (true for this problem's inputs)
    if np.any(bg) or np.any(be):
        return _numpy_fallback(x, Wg, bg, We, be)

    if "nc" not in _CACHED:
        _CACHED["nc"] = _build_nc()
    nc = _CACHED["nc"]

    in_maps = prepare_in_maps(x, Wg, bg, We, be)

    global LAST_RESULTS
    res = run_bass_kernel_spmd(nc, in_maps, list(range(N_CORES)))
    LAST_RESULTS = res
    out = np.stack([np.asarray(res.results[c]["O"]) for c in range(N_CORES)],
                   axis=0)
    if OUT_W != U:
        # gather-time fold of the remaining expert groups (fast bf16 ->
        # fp32 upcast via bit trick, then reduce)
        if out.dtype != np.float32:
            out = (out.view(np.uint16).astype(np.uint32) << 16).view(
                np.float32)
        out = out.reshape(N_CORES, TOK, OUT_W // U, U).sum(axis=2)
    return out.reshape(B, T, U).astype(np.float32)
